# revision 10
# baseline (speedup 1.0000x reference)
"""Trainium2 Bass kernel for nn_ConsistencyLoss (BCE + dilated-stencil consistency loss).

loss = mean( unfolded_weights * thred + bce )
  bce      = -(y_true*max(log(y_pred),-100) + (1-y_true)*max(log1p(-y_pred),-100))
  unfolded = max over 8 dilated (DIL=2) neighbors nb of |y_pred - nb|, zero-padded
  thred    = y_pred * (y_pred >= 0.5)

Strategy (8 NeuronCores, data-parallel over batch, 2 images/core):
  - Per band of 128 rows, both images side by side in [128, 2048] tiles.
  - unfolded = max(c - nmin, nmax - c) where nmax/nmin are separable
    min/max over the dilated 3x3 window INCLUDING the center (including the
    center never changes the result since |c-c|=0 <= unfolded).
  - Vertical (partition-dim) shifts via SBUF->SBUF DMA copies; horizontal
    shifts via free-dim slices of zero-padded tiles. All stencil math in bf16
    on the DVE (2x mode).
  - BCE logs on the Scalar engine (ACT): ln(x + tiny) reproduces torch's
    -100 clamp exactly for uniform inputs (only x==0 can clamp, and
    ln(1.2e-38) = -87.3; error contribution ~1e-6 relative, and the tiny
    bias is invisible for any x >= 2^-24).
  - All product-sums (U*R, U*m, yt*lp, yt*l1p) via TensorE "diagonal"
    matmuls accumulated in PSUM; sum(l1p) rides the ACT pass accum_out.
  - Host extracts PSUM diagonals and assembles the scalar in float64.
"""

from contextlib import ExitStack

import numpy as np

import concourse.bacc as bacc
import concourse.tile as tile
from concourse import mybir
from concourse.bass_utils import run_bass_kernel_spmd

F32 = mybir.dt.float32
BF16 = mybir.dt.bfloat16
OP = mybir.AluOpType
AT = mybir.ActivationFunctionType

B, H, W = 16, 1024, 1024
NCORES = 8
IPC = B // NCORES          # images per core = 2
P = 128                    # partitions
NBANDS = H // P            # 8 bands of 128 rows per image
FW = IPC * W               # 2048: [img0 row-block | img1 row-block]
BW = W + 4                 # 1028: per-image padded block width (2 zero cols each side)
DIL = 2
TINY = 1.18e-38            # min normal fp32; ln(x+TINY) == ln(x) for x >= 2^-24

N_OUT = 2 * 256 + NBANDS   # psumA, psumB, l1p accums


def _pad3(t):
    """[P, 2*BW] padded tile -> [P, 2, BW] view."""
    return t.rearrange("p (b w) -> p b w", b=2)


def _blk3(t):
    """[P, FW] tile -> [P, 2, W] view."""
    return t.rearrange("p (b w) -> p b w", b=2)


def _kernel_body(ctx, tc, yp, yt, out):
    nc = tc.nc

    xpool = ctx.enter_context(tc.tile_pool(name="xpool", bufs=2))
    xbpool = ctx.enter_context(tc.tile_pool(name="xbpool", bufs=4))
    ytpool = ctx.enter_context(tc.tile_pool(name="ytpool", bufs=2))
    fpool = ctx.enter_context(tc.tile_pool(name="fpool", bufs=2))    # lpl1p / rm
    shpool = ctx.enter_context(tc.tile_pool(name="shpool", bufs=2))  # xu/xd
    vpool = ctx.enter_context(tc.tile_pool(name="vpool", bufs=2))    # padded vmax/vmin
    spool = ctx.enter_context(tc.tile_pool(name="spool", bufs=1))    # stencil temps
    upool = ctx.enter_context(tc.tile_pool(name="upool", bufs=2))
    single = ctx.enter_context(tc.tile_pool(name="single", bufs=1))
    psum = ctx.enter_context(tc.tile_pool(name="psum", bufs=1, space="PSUM"))

    l1pacc = single.tile([P, NBANDS], F32)
    psum_a = psum.tile([P, 256], F32)
    psum_b = psum.tile([P, 256], F32)

    bias_tiny = single.tile([P, 1], F32)
    nc.gpsimd.memset(bias_tiny, TINY)
    bias_one = single.tile([P, 1], F32)
    nc.gpsimd.memset(bias_one, 1.0)
    bias_neghalf = single.tile([P, 1], F32)
    nc.gpsimd.memset(bias_neghalf, -0.5)

    zrow = single.tile([DIL, FW], BF16)
    nc.vector.memset(zrow, 0.0)

    xb_tiles = {}
    rm_tiles = {}

    n_pieces = FW // P  # 16 lhsT pieces per band

    def load_band(t):
        # fp32 band (both images) for ACT/GPSIMD consumers
        x = xpool.tile([P, FW], F32, name=f"x_{t}", tag="x")
        nc.sync.dma_start(out=x[:, 0:W], in_=yp[0, t * P:(t + 1) * P, :])
        nc.sync.dma_start(out=x[:, W:FW], in_=yp[1, t * P:(t + 1) * P, :])

        # bf16 copy of the band (stencil source)
        xb = xbpool.tile([P, FW], BF16, name=f"xb_{t}", tag="xb")
        nc.gpsimd.tensor_copy(out=xb, in_=x)
        xb_tiles[t] = xb

        # y_true, cast to bf16 during the HBM load
        ytb = ytpool.tile([P, FW], BF16, name=f"ytb_{t}", tag="ytb")
        nc.gpsimd.dma_start(out=ytb[:, 0:W], in_=yt[0, t * P:(t + 1) * P, :])
        nc.gpsimd.dma_start(out=ytb[:, W:FW], in_=yt[1, t * P:(t + 1) * P, :])

        # logs on ACT: lp = ln(x+TINY), l1p = ln(1-x) (+ running sum of l1p)
        lpl1p = fpool.tile([P, 2 * FW], BF16, name=f"lpl1p_{t}", tag="lpl1p")
        nc.scalar.activation(lpl1p[:, 0:FW], x, AT.Ln, bias=bias_tiny, scale=1.0)
        nc.scalar.activation(
            lpl1p[:, FW:2 * FW], x, AT.Ln, bias=bias_one, scale=-1.0,
            accum_out=l1pacc[:, t:t + 1],
        )

        # R = relu(x-0.5) on ACT ; m = (x >= 0.5) on GPSIMD
        rm = fpool.tile([P, 2 * FW], BF16, name=f"rm_{t}", tag="rm")
        nc.scalar.activation(rm[:, 0:FW], x, AT.Relu, bias=bias_neghalf, scale=1.0)
        nc.gpsimd.tensor_scalar(
            out=rm[:, FW:2 * FW], in0=x, scalar1=0.5, scalar2=None, op0=OP.is_ge,
        )
        rm_tiles[t] = rm

        # BCE product-sums: psum_b[m, n] += sum_k ytb[k, m] * [lp|l1p][k, n]
        lp3 = lpl1p.rearrange("p (b w) -> p b w", b=2)
        for j in range(n_pieces):
            nc.tensor.matmul(
                psum_b,
                ytb[:, j * P:(j + 1) * P],
                lp3[:, :, j * P:(j + 1) * P],
                start=(t == 0 and j == 0),
                stop=(t == NBANDS - 1 and j == n_pieces - 1),
            )

    def stencil_band(b):
        xbc = xb_tiles[b]

        # vertically shifted copies (partition shift +-2) with zero fill at
        # the image top/bottom
        xu = shpool.tile([P, FW], BF16, name=f"xu_{b}", tag="xu")
        xd = shpool.tile([P, FW], BF16, name=f"xd_{b}", tag="xd")
        nc.sync.dma_start(out=xu[0:P - DIL, :], in_=xbc[DIL:P, :])
        if b + 1 < NBANDS:
            nc.sync.dma_start(out=xu[P - DIL:P, :], in_=xb_tiles[b + 1][0:DIL, :])
        else:
            nc.sync.dma_start(out=xu[P - DIL:P, :], in_=zrow)
        nc.sync.dma_start(out=xd[DIL:P, :], in_=xbc[0:P - DIL, :])
        if b - 1 >= 0:
            nc.sync.dma_start(out=xd[0:DIL, :], in_=xb_tiles[b - 1][P - DIL:P, :])
        else:
            nc.vector.memset(xd[0:DIL, :], 0.0)

        # vertical 3-max / 3-min into zero-padded tiles
        vmax = vpool.tile([P, 2 * BW], BF16, name=f"vmax_{b}", tag="vmax")
        vmin = vpool.tile([P, 2 * BW], BF16, name=f"vmin_{b}", tag="vmin")
        for v in (vmax, vmin):
            nc.gpsimd.memset(v[:, 0:2], 0.0)
            nc.gpsimd.memset(v[:, BW - 2:BW + 2], 0.0)
            nc.gpsimd.memset(v[:, 2 * BW - 2:2 * BW], 0.0)
        vmax3 = _pad3(vmax)
        vmin3 = _pad3(vmin)

        va = spool.tile([P, FW], BF16, name=f"va_{b}", tag="va")
        nc.vector.tensor_tensor(out=va, in0=xu, in1=xd, op=OP.max)
        nc.vector.tensor_tensor(
            out=vmax3[:, :, 2:2 + W], in0=_blk3(va), in1=_blk3(xbc), op=OP.max)
        vb = spool.tile([P, FW], BF16, name=f"vb_{b}", tag="vb")
        nc.vector.tensor_tensor(out=vb, in0=xu, in1=xd, op=OP.min)
        nc.vector.tensor_tensor(
            out=vmin3[:, :, 2:2 + W], in0=_blk3(vb), in1=_blk3(xbc), op=OP.min)

        # horizontal dilated 3-max / 3-min
        nxa = spool.tile([P, FW], BF16, name=f"nxa_{b}", tag="nxa")
        nc.vector.tensor_tensor(
            out=_blk3(nxa), in0=vmax3[:, :, 0:W], in1=vmax3[:, :, 4:4 + W], op=OP.max)
        nx = spool.tile([P, FW], BF16, name=f"nx_{b}", tag="nx")
        nc.vector.tensor_tensor(
            out=_blk3(nx), in0=_blk3(nxa), in1=vmax3[:, :, 2:2 + W], op=OP.max)
        nma = spool.tile([P, FW], BF16, name=f"nma_{b}", tag="nma")
        nc.vector.tensor_tensor(
            out=_blk3(nma), in0=vmin3[:, :, 0:W], in1=vmin3[:, :, 4:4 + W], op=OP.min)
        nm = spool.tile([P, FW], BF16, name=f"nm_{b}", tag="nm")
        nc.vector.tensor_tensor(
            out=_blk3(nm), in0=_blk3(nma), in1=vmin3[:, :, 2:2 + W], op=OP.min)

        # unfolded = max(xb - nmin, nmax - xb)
        u1 = spool.tile([P, FW], BF16, name=f"u1_{b}", tag="u1")
        nc.vector.tensor_tensor(out=u1, in0=xbc, in1=nm, op=OP.subtract)
        u2 = spool.tile([P, FW], BF16, name=f"u2_{b}", tag="u2")
        nc.vector.tensor_tensor(out=u2, in0=nx, in1=xbc, op=OP.subtract)
        u = upool.tile([P, FW], BF16, name=f"u_{b}", tag="u")
        nc.vector.tensor_tensor(out=u, in0=u1, in1=u2, op=OP.max)

        # psum_a[m, n] += sum_k u[k, m] * [R|m][k, n]
        rm3 = rm_tiles[b].rearrange("p (b w) -> p b w", b=2)
        for j in range(n_pieces):
            nc.tensor.matmul(
                psum_a,
                u[:, j * P:(j + 1) * P],
                rm3[:, :, j * P:(j + 1) * P],
                start=(b == 0 and j == 0),
                stop=(b == NBANDS - 1 and j == n_pieces - 1),
            )

    # software pipeline: load band t while running the stencil on band t-1
    for t in range(NBANDS + 1):
        if t < NBANDS:
            load_band(t)
        if t >= 1:
            stencil_band(t - 1)

    # PSUM -> SBUF -> DRAM
    res = single.tile([P, 512], F32)
    nc.vector.tensor_copy(out=res[:, 0:256], in_=psum_a)
    nc.vector.tensor_copy(out=res[:, 256:512], in_=psum_b)
    nc.sync.dma_start(out=out[:, 0:512], in_=res)
    nc.sync.dma_start(out=out[:, 512:512 + NBANDS], in_=l1pacc)


_CACHED = {}


def _build():
    if "nc" in _CACHED:
        return _CACHED["nc"]
    nc = bacc.Bacc(
        "TRN2",
        target_bir_lowering=False,
        debug=False,
        num_devices=NCORES,
    )
    yp = nc.dram_tensor("y_pred", [IPC, H, W], F32, kind="ExternalInput").ap()
    yt = nc.dram_tensor("y_true", [IPC, H, W], F32, kind="ExternalInput").ap()
    out = nc.dram_tensor("out", [P, N_OUT], F32, kind="ExternalOutput").ap()
    with tile.TileContext(nc) as tc:
        with ExitStack() as ctx:
            _kernel_body(ctx, tc, yp, yt, out)
    nc.compile()
    _CACHED["nc"] = nc
    return nc


def _host_reduce(outs):
    """Assemble the scalar loss from the 8 per-core [P, N_OUT] partial tensors."""
    total = np.float64(0.0)
    idx = np.arange(P)
    for o in outs:
        o = np.asarray(o, dtype=np.float64)
        a, bq, l1 = o[:, 0:256], o[:, 256:512], o[:, 512:512 + NBANDS]
        sum_ur = a[idx, idx].sum()          # sum U * relu(x-.5)
        sum_um = a[idx, 128 + idx].sum()    # sum U * (x>=.5)
        sum_ylp = bq[idx, idx].sum()        # sum yt * ln(x)
        sum_yl1p = bq[idx, 128 + idx].sum() # sum yt * ln(1-x)
        sum_l1p = l1.sum()                  # sum ln(1-x)
        total += (sum_ur + 0.5 * sum_um) - sum_ylp - sum_l1p + sum_yl1p
    return np.float32(total / (B * H * W))


def kernel(y_true, y_pred):
    y_true = np.ascontiguousarray(np.asarray(y_true, dtype=np.float32)).reshape(B, H, W)
    y_pred = np.ascontiguousarray(np.asarray(y_pred, dtype=np.float32)).reshape(B, H, W)

    nc = _build()
    in_maps = []
    for r in range(NCORES):
        in_maps.append({
            "y_pred": np.ascontiguousarray(y_pred[r * IPC:(r + 1) * IPC]),
            "y_true": np.ascontiguousarray(y_true[r * IPC:(r + 1) * IPC]),
        })
    res = run_bass_kernel_spmd(nc, in_maps, core_ids=list(range(NCORES)))
    outs = [res.results[r]["out"] for r in range(NCORES)]
    return _host_reduce(outs)


# revision 11
# speedup vs baseline: 2.6441x; 2.6441x over previous
"""Trainium2 Bass kernel for nn_ConsistencyLoss (BCE + dilated-stencil consistency loss).

loss = mean( unfolded_weights * thred + bce )
  bce      = -(y_true*max(log(y_pred),-100) + (1-y_true)*max(log1p(-y_pred),-100))
  unfolded = max over 8 dilated (DIL=2) neighbors nb of |y_pred - nb|, zero-padded
  thred    = y_pred * (y_pred >= 0.5)

Strategy (8 NeuronCores, data-parallel over batch, 2 images/core):
  - Per band of 128 rows, both images side by side in [128, 2048] tiles.
  - unfolded = max(c - nmin, nmax - c) where nmax/nmin are separable
    min/max over the dilated 3x3 window INCLUDING the center (including the
    center never changes the result since |c-c|=0 <= unfolded).
  - Vertical (partition-dim) shifts via SBUF->SBUF DMA copies; horizontal
    shifts via free-dim slices of zero-padded tiles. All stencil math in bf16
    on the DVE (2x mode).
  - BCE logs on the Scalar engine (ACT): ln(x + tiny) reproduces torch's
    -100 clamp exactly for uniform inputs (only x==0 can clamp, and
    ln(1.2e-38) = -87.3; error contribution ~1e-6 relative, and the tiny
    bias is invisible for any x >= 2^-24).
  - All product-sums (U*R, U*m, yt*lp, yt*l1p) via TensorE "diagonal"
    matmuls accumulated in PSUM; sum(l1p) rides the ACT pass accum_out.
  - Host extracts PSUM diagonals and assembles the scalar in float64.
"""

from contextlib import ExitStack

import numpy as np

import concourse.bacc as bacc
import concourse.tile as tile
from concourse import mybir
from concourse.bass_utils import run_bass_kernel_spmd

F32 = mybir.dt.float32
BF16 = mybir.dt.bfloat16
OP = mybir.AluOpType
AT = mybir.ActivationFunctionType

B, H, W = 16, 1024, 1024
NCORES = 8
IPC = B // NCORES          # images per core = 2
P = 128                    # partitions
NBANDS = H // P            # 8 bands of 128 rows per image
FW = IPC * W               # 2048: [img0 row-block | img1 row-block]
BW = W + 4                 # 1028: per-image padded block width (2 zero cols each side)
DIL = 2
TINY = 1.18e-38            # min normal fp32; ln(x+TINY) == ln(x) for x >= 2^-24

N_OUT = 2 * 256 + NBANDS   # psumA, psumB, l1p accums


def _pad3(t):
    """[P, 2*BW] padded tile -> [P, 2, BW] view."""
    return t.rearrange("p (b w) -> p b w", b=2)


def _blk3(t):
    """[P, FW] tile -> [P, 2, W] view."""
    return t.rearrange("p (b w) -> p b w", b=2)


def _kernel_body(ctx, tc, yp, yt, out):
    nc = tc.nc

    xpool = ctx.enter_context(tc.tile_pool(name="xpool", bufs=2))
    xbpool = ctx.enter_context(tc.tile_pool(name="xbpool", bufs=4))
    ytpool = ctx.enter_context(tc.tile_pool(name="ytpool", bufs=2))
    fpool = ctx.enter_context(tc.tile_pool(name="fpool", bufs=2))    # lpl1p / rm
    shpool = ctx.enter_context(tc.tile_pool(name="shpool", bufs=2))  # xu/xd
    vpool = ctx.enter_context(tc.tile_pool(name="vpool", bufs=2))    # padded vmax/vmin
    spool = ctx.enter_context(tc.tile_pool(name="spool", bufs=1))    # stencil temps
    upool = ctx.enter_context(tc.tile_pool(name="upool", bufs=2))
    single = ctx.enter_context(tc.tile_pool(name="single", bufs=1))
    psum = ctx.enter_context(tc.tile_pool(name="psum", bufs=1, space="PSUM"))

    l1pacc = single.tile([P, NBANDS], F32)
    psum_a = psum.tile([P, 256], F32)
    psum_b = psum.tile([P, 256], F32)

    bias_tiny = single.tile([P, 1], F32)
    nc.gpsimd.memset(bias_tiny, TINY)
    bias_one = single.tile([P, 1], F32)
    nc.gpsimd.memset(bias_one, 1.0)
    bias_neghalf = single.tile([P, 1], F32)
    nc.gpsimd.memset(bias_neghalf, -0.5)

    zrow = single.tile([DIL, FW], BF16)
    nc.vector.memset(zrow, 0.0)

    xb_tiles = {}
    rm_tiles = {}

    n_pieces = FW // P  # 16 lhsT pieces per band

    def load_band(t):
        # fp32 band (both images) for ACT/GPSIMD consumers
        x = xpool.tile([P, FW], F32, name=f"x_{t}", tag="x")
        nc.sync.dma_start(out=x[:, 0:W], in_=yp[0, t * P:(t + 1) * P, :])
        nc.sync.dma_start(out=x[:, W:FW], in_=yp[1, t * P:(t + 1) * P, :])

        # bf16 copy of the band (stencil source), cast during the HBM load
        xb = xbpool.tile([P, FW], BF16, name=f"xb_{t}", tag="xb")
        nc.gpsimd.dma_start(out=xb[:, 0:W], in_=yp[0, t * P:(t + 1) * P, :])
        nc.gpsimd.dma_start(out=xb[:, W:FW], in_=yp[1, t * P:(t + 1) * P, :])
        xb_tiles[t] = xb

        # y_true, cast to bf16 during the HBM load
        ytb = ytpool.tile([P, FW], BF16, name=f"ytb_{t}", tag="ytb")
        nc.gpsimd.dma_start(out=ytb[:, 0:W], in_=yt[0, t * P:(t + 1) * P, :])
        nc.gpsimd.dma_start(out=ytb[:, W:FW], in_=yt[1, t * P:(t + 1) * P, :])

        # logs on ACT: lp = ln(x+TINY), l1p = ln(1-x) (+ running sum of l1p)
        lpl1p = fpool.tile([P, 2 * FW], BF16, name=f"lpl1p_{t}", tag="lpl1p")
        nc.scalar.activation(lpl1p[:, 0:FW], x, AT.Ln, bias=bias_tiny, scale=1.0)
        nc.scalar.activation(
            lpl1p[:, FW:2 * FW], x, AT.Ln, bias=bias_one, scale=-1.0,
            accum_out=l1pacc[:, t:t + 1],
        )

        # R = relu(x-0.5) on ACT ; m = (x >= 0.5) on DVE (tensor_scalar, 2x)
        rm = fpool.tile([P, 2 * FW], BF16, name=f"rm_{t}", tag="rm")
        nc.scalar.activation(rm[:, 0:FW], x, AT.Relu, bias=bias_neghalf, scale=1.0)
        nc.vector.tensor_scalar(
            out=rm[:, FW:2 * FW], in0=x, scalar1=0.5, scalar2=None, op0=OP.is_ge,
        )
        rm_tiles[t] = rm

        # BCE product-sums: psum_b[m, n] += sum_k ytb[k, m] * [lp|l1p][k, n]
        lp3 = lpl1p.rearrange("p (b w) -> p b w", b=2)
        for j in range(n_pieces):
            nc.tensor.matmul(
                psum_b,
                ytb[:, j * P:(j + 1) * P],
                lp3[:, :, j * P:(j + 1) * P],
                start=(t == 0 and j == 0),
                stop=(t == NBANDS - 1 and j == n_pieces - 1),
            )

    def stencil_band(b):
        xbc = xb_tiles[b]

        # vertically shifted copies (partition shift +-2) with zero fill at
        # the image top/bottom
        xu = shpool.tile([P, FW], BF16, name=f"xu_{b}", tag="xu")
        xd = shpool.tile([P, FW], BF16, name=f"xd_{b}", tag="xd")
        nc.sync.dma_start(out=xu[0:P - DIL, :], in_=xbc[DIL:P, :])
        if b + 1 < NBANDS:
            nc.sync.dma_start(out=xu[P - DIL:P, :], in_=xb_tiles[b + 1][0:DIL, :])
        else:
            nc.sync.dma_start(out=xu[P - DIL:P, :], in_=zrow)
        nc.sync.dma_start(out=xd[DIL:P, :], in_=xbc[0:P - DIL, :])
        if b - 1 >= 0:
            nc.sync.dma_start(out=xd[0:DIL, :], in_=xb_tiles[b - 1][P - DIL:P, :])
        else:
            nc.vector.memset(xd[0:DIL, :], 0.0)

        # vertical 3-max / 3-min into zero-padded tiles
        vmax = vpool.tile([P, 2 * BW], BF16, name=f"vmax_{b}", tag="vmax")
        vmin = vpool.tile([P, 2 * BW], BF16, name=f"vmin_{b}", tag="vmin")
        for v in (vmax, vmin):
            nc.gpsimd.memset(v[:, 0:2], 0.0)
            nc.gpsimd.memset(v[:, BW - 2:BW + 2], 0.0)
            nc.gpsimd.memset(v[:, 2 * BW - 2:2 * BW], 0.0)
        vmax3 = _pad3(vmax)
        vmin3 = _pad3(vmin)

        va = spool.tile([P, FW], BF16, name=f"va_{b}", tag="va")
        nc.vector.tensor_tensor(out=va, in0=xu, in1=xd, op=OP.max)
        nc.vector.tensor_tensor(
            out=vmax3[:, :, 2:2 + W], in0=_blk3(va), in1=_blk3(xbc), op=OP.max)
        vb = spool.tile([P, FW], BF16, name=f"vb_{b}", tag="vb")
        nc.vector.tensor_tensor(out=vb, in0=xu, in1=xd, op=OP.min)
        nc.vector.tensor_tensor(
            out=vmin3[:, :, 2:2 + W], in0=_blk3(vb), in1=_blk3(xbc), op=OP.min)

        # horizontal dilated 3-max / 3-min
        nxa = spool.tile([P, FW], BF16, name=f"nxa_{b}", tag="nxa")
        nc.vector.tensor_tensor(
            out=_blk3(nxa), in0=vmax3[:, :, 0:W], in1=vmax3[:, :, 4:4 + W], op=OP.max)
        nx = spool.tile([P, FW], BF16, name=f"nx_{b}", tag="nx")
        nc.vector.tensor_tensor(
            out=_blk3(nx), in0=_blk3(nxa), in1=vmax3[:, :, 2:2 + W], op=OP.max)
        nma = spool.tile([P, FW], BF16, name=f"nma_{b}", tag="nma")
        nc.vector.tensor_tensor(
            out=_blk3(nma), in0=vmin3[:, :, 0:W], in1=vmin3[:, :, 4:4 + W], op=OP.min)
        nm = spool.tile([P, FW], BF16, name=f"nm_{b}", tag="nm")
        nc.vector.tensor_tensor(
            out=_blk3(nm), in0=_blk3(nma), in1=vmin3[:, :, 2:2 + W], op=OP.min)

        # unfolded = max(xb - nmin, nmax - xb)
        u1 = spool.tile([P, FW], BF16, name=f"u1_{b}", tag="u1")
        nc.vector.tensor_tensor(out=u1, in0=xbc, in1=nm, op=OP.subtract)
        u2 = spool.tile([P, FW], BF16, name=f"u2_{b}", tag="u2")
        nc.vector.tensor_tensor(out=u2, in0=nx, in1=xbc, op=OP.subtract)
        u = upool.tile([P, FW], BF16, name=f"u_{b}", tag="u")
        nc.vector.tensor_tensor(out=u, in0=u1, in1=u2, op=OP.max)

        # psum_a[m, n] += sum_k u[k, m] * [R|m][k, n]
        rm3 = rm_tiles[b].rearrange("p (b w) -> p b w", b=2)
        for j in range(n_pieces):
            nc.tensor.matmul(
                psum_a,
                u[:, j * P:(j + 1) * P],
                rm3[:, :, j * P:(j + 1) * P],
                start=(b == 0 and j == 0),
                stop=(b == NBANDS - 1 and j == n_pieces - 1),
            )

    # software pipeline: load band t while running the stencil on band t-1
    for t in range(NBANDS + 1):
        if t < NBANDS:
            load_band(t)
        if t >= 1:
            stencil_band(t - 1)

    # PSUM -> SBUF -> DRAM
    res = single.tile([P, 512], F32)
    nc.vector.tensor_copy(out=res[:, 0:256], in_=psum_a)
    nc.vector.tensor_copy(out=res[:, 256:512], in_=psum_b)
    nc.sync.dma_start(out=out[:, 0:512], in_=res)
    nc.sync.dma_start(out=out[:, 512:512 + NBANDS], in_=l1pacc)


_CACHED = {}


def _build():
    if "nc" in _CACHED:
        return _CACHED["nc"]
    nc = bacc.Bacc(
        "TRN2",
        target_bir_lowering=False,
        debug=False,
        num_devices=NCORES,
    )
    yp = nc.dram_tensor("y_pred", [IPC, H, W], F32, kind="ExternalInput").ap()
    yt = nc.dram_tensor("y_true", [IPC, H, W], F32, kind="ExternalInput").ap()
    out = nc.dram_tensor("out", [P, N_OUT], F32, kind="ExternalOutput").ap()
    with tile.TileContext(nc) as tc:
        with ExitStack() as ctx:
            _kernel_body(ctx, tc, yp, yt, out)
    nc.compile()
    _CACHED["nc"] = nc
    return nc


def _host_reduce(outs):
    """Assemble the scalar loss from the 8 per-core [P, N_OUT] partial tensors."""
    total = np.float64(0.0)
    idx = np.arange(P)
    for o in outs:
        o = np.asarray(o, dtype=np.float64)
        a, bq, l1 = o[:, 0:256], o[:, 256:512], o[:, 512:512 + NBANDS]
        sum_ur = a[idx, idx].sum()          # sum U * relu(x-.5)
        sum_um = a[idx, 128 + idx].sum()    # sum U * (x>=.5)
        sum_ylp = bq[idx, idx].sum()        # sum yt * ln(x)
        sum_yl1p = bq[idx, 128 + idx].sum() # sum yt * ln(1-x)
        sum_l1p = l1.sum()                  # sum ln(1-x)
        total += (sum_ur + 0.5 * sum_um) - sum_ylp - sum_l1p + sum_yl1p
    return np.float32(total / (B * H * W))


def kernel(y_true, y_pred):
    y_true = np.ascontiguousarray(np.asarray(y_true, dtype=np.float32)).reshape(B, H, W)
    y_pred = np.ascontiguousarray(np.asarray(y_pred, dtype=np.float32)).reshape(B, H, W)

    nc = _build()
    in_maps = []
    for r in range(NCORES):
        in_maps.append({
            "y_pred": np.ascontiguousarray(y_pred[r * IPC:(r + 1) * IPC]),
            "y_true": np.ascontiguousarray(y_true[r * IPC:(r + 1) * IPC]),
        })
    res = run_bass_kernel_spmd(nc, in_maps, core_ids=list(range(NCORES)))
    outs = [res.results[r]["out"] for r in range(NCORES)]
    return _host_reduce(outs)
